# revision 13
# baseline (speedup 1.0000x reference)
"""Trainium2 Bass/Tile kernel: 2-layer bidirectional LSTM encoder.

Contract: kernel(**inputs) takes the FULL unsharded inputs (see shapes
below) and returns the full [T, B, 2H] output. Internally the batch is
split across 8 NeuronCores (data parallel); weights are replicated.

Shapes (hardcoded): T=160, B=256, C=512, H=256, G=4H=1024, 8 cores,
BC = 32 batch per core.

Per-core algorithm (both directions interleaved per step):
 - xg = x @ Wih.T + bias computed as a chunked GEMM ("quads" of 4
   timesteps -> PSUM [128=(4t,32b), 1024]), fp32r matmuls, bias folded
   in with a K=1 ones-row matmul, then evacuated to SBUF.
 - scan: gates [64=(2dir,32b), 1024] accumulate in PSUM:
   identity-matmul injects xg (start=True), then two K=128 recurrent
   matmuls vs Whh.T accumulate.  Gate order is host-permuted to
   [i,f,o,g] so one sigmoid covers cols 0:768 and one tanh 768:1024.
 - c = f*c + i*g ; h = o*tanh(c) on DVE/ACT.
 - h is transposed back to feature-major via PE transpose into an SBUF
   history buffer (h0T) which feeds both the next step's recurrent
   matmul and layer 1's xg GEMM.
"""

import os
import sys

import numpy as np

for _p in ("/opt/trn_rl_repo", "/root/.axon_site/_ro/trn_rl_repo"):
    if os.path.isdir(_p) and _p not in sys.path:
        sys.path.insert(0, _p)

from contextlib import ExitStack

import concourse.bass as bass  # noqa: F401
import concourse.mybir as mybir
import concourse.tile as tile
from concourse import bacc, bass_utils

AF = mybir.ActivationFunctionType
F32 = mybir.dt.float32
F32R = mybir.dt.float32r
BF16 = mybir.dt.bfloat16

T, B, CIN, H = 160, 256, 512, 256
G = 4 * H  # 1024
HALF = G // 2  # 512
NCORES = 8
BC = B // NCORES  # 32
NQ = T // 4  # 40 quads of 4 timesteps

# torch gate order [i,f,g,o] -> ours [i,f,o,g] (sigmoid block contiguous)
_PERM = np.concatenate(
    [np.arange(0, 512), np.arange(768, 1024), np.arange(512, 768)]
)

_CACHE = {}


def _r(ap):
    return ap


def _build():
    nc = bacc.Bacc("TRN2", target_bir_lowering=False, debug=False)

    xT_d = [
        nc.dram_tensor(f"xT{d}", [CIN, T * BC], F32R, kind="ExternalInput").ap()
        for d in (0, 1)
    ]
    wih_d = [
        [
            nc.dram_tensor(f"wih{l}{d}", [128, 4 * G], F32R if l == 0 else BF16, kind="ExternalInput").ap()
            for d in (0, 1)
        ]
        for l in (0, 1)
    ]
    whh_d = [
        [
            nc.dram_tensor(f"whh{l}{d}", [128, 2 * G], BF16, kind="ExternalInput").ap()
            for d in (0, 1)
        ]
        for l in (0, 1)
    ]
    bias_d = [
        [
            nc.dram_tensor(f"bias{l}{d}", [1, G], F32R, kind="ExternalInput").ap()
            for d in (0, 1)
        ]
        for l in (0, 1)
    ]
    ident4_d = nc.dram_tensor("ident4", [128, 32], BF16, kind="ExternalInput").ap()
    ones_d = nc.dram_tensor("ones", [1, 128], F32R, kind="ExternalInput").ap()
    identT_d = nc.dram_tensor("identT", [64, 64], F32R, kind="ExternalInput").ap()
    out_d = nc.dram_tensor("out", [T, BC, 2 * H], F32, kind="ExternalOutput").ap()

    with tile.TileContext(nc) as tc, ExitStack() as ctx:
        sb = ctx.enter_context(tc.tile_pool(name="sb", bufs=2))
        const = ctx.enter_context(tc.tile_pool(name="const", bufs=1))
        big = ctx.enter_context(tc.tile_pool(name="big", bufs=1))
        ps_xg = ctx.enter_context(tc.tile_pool(name="ps_xg", bufs=1, space="PSUM"))
        ps_g = ctx.enter_context(tc.tile_pool(name="ps_g", bufs=2, space="PSUM"))
        ps_t = ctx.enter_context(tc.tile_pool(name="ps_t", bufs=2, space="PSUM"))

        ident4_sb = const.tile([128, 32], BF16)
        identT_sb = const.tile([64, 64], F32R)
        nc.sync.dma_start(identT_sb[:], identT_d[:])
        nc.sync.dma_start(ident4_sb[:], ident4_d[:])
        ones_sb = const.tile([1, 128], F32R)
        nc.sync.dma_start(ones_sb[:], ones_d[:])

        # h0T: layer-0 output, feature-major: [128, (k=2, dir=2, t=T, b=32)]
        h0T = big.tile([128, 2 * T * 64], BF16)
        h0T_r = h0T[:].rearrange("p (k dd t b) -> p k dd t b", k=2, dd=2, t=T)

        for l in (0, 1):
            wih_sb = [
                sb.tile([128, 4 * G], F32R if l == 0 else BF16, tag=f"wih{d}", bufs=1, name=f"wih{l}{d}s")
                for d in (0, 1)
            ]
            whh_sb = [
                sb.tile([128, 2 * G], BF16, tag=f"whh{d}", bufs=1, name=f"whh{l}{d}s")
                for d in (0, 1)
            ]
            bias_sb = [
                sb.tile([1, G], F32R, tag=f"bias{d}", bufs=1, name=f"bias{l}{d}s")
                for d in (0, 1)
            ]
            for d in (0, 1):
                nc.sync.dma_start(wih_sb[d][:], wih_d[l][d][:])
                nc.sync.dma_start(whh_sb[d][:], whh_d[l][d][:])
                nc.sync.dma_start(bias_sb[d][:], bias_d[l][d][:])

            xg_live = {}

            def emit_gemm(q, d, l=l, wih_sb=wih_sb, bias_sb=bias_sb, xg_live=xg_live):
                xg_ps = ps_xg.tile([128, G], F32, tag="xgps")
                for ki in range(4):
                    if l == 0:
                        stat = sb.tile([128, 128], F32R, tag="xstat", bufs=4)
                        nc.sync.dma_start(
                            stat[:],
                            xT_d[d][ki * 128 : (ki + 1) * 128, q * 128 : (q + 1) * 128],
                        )
                        lhsT = stat[:]
                    else:
                        t0 = 4 * q if d == 0 else T - 4 - 4 * q
                        k, dsrc = ki % 2, ki // 2
                        base = k * 10240 + dsrc * 5120 + t0 * 32
                        lhsT = h0T[:, base : base + 128]
                    for nh in (0, 1):
                        nc.tensor.matmul(
                            xg_ps[:, nh * HALF : (nh + 1) * HALF],
                            _r(lhsT),
                            _r(
                                wih_sb[d][
                                    :, ki * G + nh * HALF : ki * G + (nh + 1) * HALF
                                ]
                            ),
                            start=(ki == 0),
                            stop=False,
                        )
                for nh in (0, 1):
                    nc.tensor.matmul(
                        xg_ps[:, nh * HALF : (nh + 1) * HALF],
                        _r(ones_sb[:]),
                        _r(bias_sb[d][:, nh * HALF : (nh + 1) * HALF]),
                        start=False,
                        stop=True,
                    )
                xg_t = sb.tile([128, G], F32, tag=f"xg{d}", bufs=2)
                nc.scalar.activation(xg_t[:], xg_ps[:], AF.Copy)
                xg_live[(d, q)] = xg_t

            for q0 in (0, 1):
                emit_gemm(q0, 0)
                emit_gemm(q0, 1)

            c_prev = None
            hT_prev = None
            for s in range(T):
                q, r = divmod(s, 4)
                if r == 0 and q + 2 < NQ:
                    emit_gemm(q + 2, 0)
                    emit_gemm(q + 2, 1)

                xgs = sb.tile([64, G], F32, tag="xgs", bufs=3)
                for d in (0, 1):
                    slot = r if (l == 0 or d == 0) else 3 - r
                    nc.sync.dma_start(
                        xgs[32 * d : 32 * d + 32, :],
                        xg_live[(d, q)][32 * slot : 32 * slot + 32, :],
                    )
                if s > 0:
                    gates = ps_g.tile([64, G], F32, tag="gates", bufs=2)
                    for d in (0, 1):
                        grows = slice(32 * d, 32 * d + 32)
                        for k in (0, 1):
                            if l == 0:
                                t_prev = (s - 1) if d == 0 else T - s
                                lhsT_h = h0T_r[:, k, d, t_prev, :]
                            else:
                                lhsT_h = hT_prev[:, k * 64 + 32 * d : k * 64 + 32 * d + 32]
                            for nh in (0, 1):
                                nc.tensor.matmul(
                                    gates[grows, nh * HALF : (nh + 1) * HALF],
                                    _r(lhsT_h),
                                    _r(
                                        whh_sb[d][
                                            :,
                                            k * G + nh * HALF : k * G + (nh + 1) * HALF,
                                        ]
                                    ),
                                    start=(k == 0),
                                    stop=(k == 1),
                                    tile_position=(0, 32 * d),
                                )
                    gsum = sb.tile([64, G], F32, tag="gsum", bufs=2)
                    nc.vector.tensor_add(gsum[:], gates[:], xgs[:])
                else:
                    gsum = xgs

                ifo = sb.tile([64, 768], F32, tag="ifo")
                nc.scalar.activation(ifo[:], gsum[:, 0:768], AF.Sigmoid)
                gt = sb.tile([64, 256], F32, tag="gt")
                nc.scalar.activation(gt[:], gsum[:, 768:1024], AF.Tanh)
                ig = sb.tile([64, 256], F32, tag="ig")
                nc.vector.tensor_mul(ig[:], ifo[:, 0:256], gt[:])
                c_new = sb.tile([64, 256], F32, tag="c", bufs=2)
                if s == 0:
                    nc.vector.tensor_copy(c_new[:], ig[:])
                else:
                    fc = sb.tile([64, 256], F32, tag="fc")
                    nc.gpsimd.tensor_mul(fc[:], ifo[:, 256:512], c_prev[:])
                    nc.vector.tensor_add(c_new[:], fc[:], ig[:])
                c_prev = c_new
                tct = sb.tile([64, 256], F32, tag="tct")
                nc.scalar.activation(tct[:], c_new[:], AF.Tanh)
                h = sb.tile([64, 256], F32R, tag="h", bufs=3)
                nc.vector.tensor_mul(h[:], ifo[:, 512:768], tct[:])

                t_f, t_b = s, T - 1 - s
                if l == 0:
                    for k in (0, 1):
                        trp = ps_t.tile([128, 64], F32R, tag="trp", bufs=2)
                        nc.tensor.transpose(
                            _r(trp[:]),
                            _r(h[:, k * 128 : (k + 1) * 128]),
                            _r(identT_sb[:]),
                        )
                        nc.vector.tensor_copy(
                            h0T_r[:, k, 0, t_f, :], trp[:, 0:32]
                        )
                        nc.vector.tensor_copy(
                            h0T_r[:, k, 1, t_b, :], trp[:, 32:64]
                        )
                else:
                    nc.sync.dma_start(out_d[t_f, :, 0:256], h[0:32, :].bitcast(F32))
                    nc.sync.dma_start(out_d[t_b, :, 256:512], h[32:64, :].bitcast(F32))
                    if s < T - 1:
                        hT_new = sb.tile([128, 128], BF16, tag="h1T", bufs=2)
                        for k in (0, 1):
                            trp = ps_t.tile([128, 64], F32R, tag="trp", bufs=2)
                            nc.tensor.transpose(
                                _r(trp[:]),
                                _r(h[:, k * 128 : (k + 1) * 128]),
                                _r(identT_sb[:]),
                            )
                            nc.vector.tensor_copy(
                                hT_new[:, k * 64 : (k + 1) * 64], trp[:]
                            )
                        hT_prev = hT_new

    nc.compile()
    return nc


def _prep_inputs(inputs):
    x = np.asarray(inputs["x"], dtype=np.float32)
    common = {}
    for l in (0, 1):
        for d, sfx in enumerate(("", "_reverse")):
            Wih = np.asarray(inputs[f"weight_ih_l{l}{sfx}"], dtype=np.float32)
            Whh = np.asarray(inputs[f"weight_hh_l{l}{sfx}"], dtype=np.float32)
            bsum = (
                np.asarray(inputs[f"bias_ih_l{l}{sfx}"], dtype=np.float32)
                + np.asarray(inputs[f"bias_hh_l{l}{sfx}"], dtype=np.float32)
            )
            wihT = np.ascontiguousarray(Wih.T[:, _PERM])  # [512, 1024]
            whhT = np.ascontiguousarray(Whh.T[:, _PERM])  # [256, 1024]
            common[f"wih{l}{d}"] = (
                wihT.reshape(4, 128, G).transpose(1, 0, 2).reshape(128, 4 * G)
            )
            common[f"whh{l}{d}"] = (
                whhT.reshape(2, 128, G).transpose(1, 0, 2).reshape(128, 2 * G)
            )
            common[f"bias{l}{d}"] = bsum[_PERM][None, :]
    common["ident4"] = np.tile(np.eye(32, dtype=np.float32), (4, 1))
    common["ones"] = np.ones((1, 128), dtype=np.float32)
    common["identT"] = np.eye(64, dtype=np.float32)
    import ml_dtypes
    bf = ml_dtypes.bfloat16
    dts = {"ident4": bf}
    for l in (0, 1):
        for d in (0, 1):
            dts[f"whh{l}{d}"] = bf
            if l == 1:
                dts[f"wih{l}{d}"] = bf
    common = {
        k: np.ascontiguousarray(v, dtype=dts.get(k, np.float32))
        for k, v in common.items()
    }

    in_maps = []
    for c in range(NCORES):
        xs = x[:, c * BC : (c + 1) * BC, :]  # [T, 32, 512]
        m = dict(common)
        m["xT0"] = np.ascontiguousarray(xs.transpose(2, 0, 1).reshape(CIN, T * BC))
        m["xT1"] = np.ascontiguousarray(
            xs[::-1].transpose(2, 0, 1).reshape(CIN, T * BC)
        )
        in_maps.append(m)
    return in_maps


def _get_program():
    if "prog" not in _CACHE:
        _CACHE["prog"] = _build()
    return _CACHE["prog"]


def kernel(**inputs):
    nc = _get_program()
    in_maps = _prep_inputs(inputs)
    res = bass_utils.run_bass_kernel_spmd(nc, in_maps, core_ids=list(range(NCORES)))
    out = np.empty((T, B, 2 * H), np.float32)
    for c in range(NCORES):
        out[:, c * BC : (c + 1) * BC, :] = res.results[c]["out"]
    return out


# revision 14
# speedup vs baseline: 1.0322x; 1.0322x over previous
"""Trainium2 Bass/Tile kernel: 2-layer bidirectional LSTM encoder.

Contract: kernel(**inputs) takes the FULL unsharded inputs (see shapes
below) and returns the full [T, B, 2H] output. Internally the batch is
split across 8 NeuronCores (data parallel); weights are replicated.

Shapes (hardcoded): T=160, B=256, C=512, H=256, G=4H=1024, 8 cores,
BC = 32 batch per core.

Per-core algorithm (both directions interleaved per step):
 - xg = x @ Wih.T + bias computed as a chunked GEMM ("quads" of 4
   timesteps -> PSUM [128=(4t,32b), 1024]), fp32r matmuls, bias folded
   in with a K=1 ones-row matmul, then evacuated to SBUF.
 - scan: gates [64=(2dir,32b), 1024] accumulate in PSUM:
   identity-matmul injects xg (start=True), then two K=128 recurrent
   matmuls vs Whh.T accumulate.  Gate order is host-permuted to
   [i,f,o,g] so one sigmoid covers cols 0:768 and one tanh 768:1024.
 - c = f*c + i*g ; h = o*tanh(c) on DVE/ACT.
 - h is transposed back to feature-major via PE transpose into an SBUF
   history buffer (h0T) which feeds both the next step's recurrent
   matmul and layer 1's xg GEMM.
"""

import os
import sys

import numpy as np

for _p in ("/opt/trn_rl_repo", "/root/.axon_site/_ro/trn_rl_repo"):
    if os.path.isdir(_p) and _p not in sys.path:
        sys.path.insert(0, _p)

from contextlib import ExitStack

import concourse.bass as bass  # noqa: F401
import concourse.mybir as mybir
import concourse.tile as tile
from concourse import bacc, bass_utils

AF = mybir.ActivationFunctionType
F32 = mybir.dt.float32
F32R = mybir.dt.float32r
BF16 = mybir.dt.bfloat16

T, B, CIN, H = 160, 256, 512, 256
G = 4 * H  # 1024
HALF = G // 2  # 512
NCORES = 8
BC = B // NCORES  # 32
NQ = T // 4  # 40 quads of 4 timesteps

# torch gate order [i,f,g,o] -> ours [i,f,o,g] (sigmoid block contiguous)
_PERM = np.concatenate(
    [np.arange(0, 512), np.arange(768, 1024), np.arange(512, 768)]
)

_CACHE = {}


def _r(ap):
    return ap


def _build():
    nc = bacc.Bacc("TRN2", target_bir_lowering=False, debug=False)

    xT_d = [
        nc.dram_tensor(f"xT{d}", [CIN, T * BC], F32R, kind="ExternalInput").ap()
        for d in (0, 1)
    ]
    wih_d = [
        [
            nc.dram_tensor(f"wih{l}{d}", [128, 4 * G], F32R if l == 0 else BF16, kind="ExternalInput").ap()
            for d in (0, 1)
        ]
        for l in (0, 1)
    ]
    whh_d = [
        [
            nc.dram_tensor(f"whh{l}{d}", [128, 2 * G], BF16, kind="ExternalInput").ap()
            for d in (0, 1)
        ]
        for l in (0, 1)
    ]
    bias_d = [
        [
            nc.dram_tensor(f"bias{l}{d}", [1, G], F32R, kind="ExternalInput").ap()
            for d in (0, 1)
        ]
        for l in (0, 1)
    ]
    ident4_d = nc.dram_tensor("ident4", [128, 32], BF16, kind="ExternalInput").ap()
    ones_d = nc.dram_tensor("ones", [1, 128], F32R, kind="ExternalInput").ap()
    identT_d = nc.dram_tensor("identT", [64, 64], F32R, kind="ExternalInput").ap()
    out_d = nc.dram_tensor("out", [T, BC, 2 * H], F32, kind="ExternalOutput").ap()

    with tile.TileContext(nc) as tc, ExitStack() as ctx:
        sb = ctx.enter_context(tc.tile_pool(name="sb", bufs=2))
        const = ctx.enter_context(tc.tile_pool(name="const", bufs=1))
        big = ctx.enter_context(tc.tile_pool(name="big", bufs=1))
        ps_xg = ctx.enter_context(tc.tile_pool(name="ps_xg", bufs=1, space="PSUM"))
        ps_g = ctx.enter_context(tc.tile_pool(name="ps_g", bufs=2, space="PSUM"))
        ps_t = ctx.enter_context(tc.tile_pool(name="ps_t", bufs=2, space="PSUM"))

        ident4_sb = const.tile([128, 32], BF16)
        identT_sb = const.tile([64, 64], F32R)
        nc.sync.dma_start(identT_sb[:], identT_d[:])
        nc.sync.dma_start(ident4_sb[:], ident4_d[:])
        ones_sb = const.tile([1, 128], F32R)
        nc.sync.dma_start(ones_sb[:], ones_d[:])

        # h0T: layer-0 output, feature-major: [128, (k=2, dir=2, t=T, b=32)]
        h0T = big.tile([128, 2 * T * 64], BF16)
        h0T_r = h0T[:].rearrange("p (k dd t b) -> p k dd t b", k=2, dd=2, t=T)

        for l in (0, 1):
            wih_sb = [
                sb.tile([128, 4 * G], F32R if l == 0 else BF16, tag=f"wih{d}", bufs=1, name=f"wih{l}{d}s")
                for d in (0, 1)
            ]
            whh_sb = [
                sb.tile([128, 2 * G], BF16, tag=f"whh{d}", bufs=1, name=f"whh{l}{d}s")
                for d in (0, 1)
            ]
            bias_sb = [
                sb.tile([1, G], F32R, tag=f"bias{d}", bufs=1, name=f"bias{l}{d}s")
                for d in (0, 1)
            ]
            for d in (0, 1):
                nc.sync.dma_start(wih_sb[d][:], wih_d[l][d][:])
                nc.sync.dma_start(whh_sb[d][:], whh_d[l][d][:])
                nc.sync.dma_start(bias_sb[d][:], bias_d[l][d][:])

            xg_live = {}

            def emit_gemm(q, d, l=l, wih_sb=wih_sb, bias_sb=bias_sb, xg_live=xg_live):
                xg_ps = ps_xg.tile([128, G], F32, tag="xgps")
                for ki in range(4):
                    if l == 0:
                        stat = sb.tile([128, 128], F32R, tag="xstat", bufs=4)
                        nc.sync.dma_start(
                            stat[:],
                            xT_d[d][ki * 128 : (ki + 1) * 128, q * 128 : (q + 1) * 128],
                        )
                        lhsT = stat[:]
                    else:
                        t0 = 4 * q if d == 0 else T - 4 - 4 * q
                        k, dsrc = ki % 2, ki // 2
                        base = k * 10240 + dsrc * 5120 + t0 * 32
                        lhsT = h0T[:, base : base + 128]
                    for nh in (0, 1):
                        nc.tensor.matmul(
                            xg_ps[:, nh * HALF : (nh + 1) * HALF],
                            _r(lhsT),
                            _r(
                                wih_sb[d][
                                    :, ki * G + nh * HALF : ki * G + (nh + 1) * HALF
                                ]
                            ),
                            start=(ki == 0),
                            stop=False,
                        )
                for nh in (0, 1):
                    nc.tensor.matmul(
                        xg_ps[:, nh * HALF : (nh + 1) * HALF],
                        _r(ones_sb[:]),
                        _r(bias_sb[d][:, nh * HALF : (nh + 1) * HALF]),
                        start=False,
                        stop=True,
                    )
                xg_t = sb.tile([128, G], F32, tag=f"xg{d}", bufs=2)
                nc.scalar.activation(xg_t[:], xg_ps[:], AF.Copy)
                xg_live[(d, q)] = xg_t

            for q0 in (0, 1):
                emit_gemm(q0, 0)
                emit_gemm(q0, 1)

            c_prev = None
            hT_prev = None
            for s in range(T):
                q, r = divmod(s, 4)
                if r == 0 and q + 2 < NQ:
                    emit_gemm(q + 2, 0)
                    emit_gemm(q + 2, 1)

                xgs = sb.tile([64, G], F32, tag="xgs", bufs=3)
                for d in (0, 1):
                    slot = r if (l == 0 or d == 0) else 3 - r
                    nc.sync.dma_start(
                        xgs[32 * d : 32 * d + 32, :],
                        xg_live[(d, q)][32 * slot : 32 * slot + 32, :],
                    )
                if s > 0:
                    gates = ps_g.tile([64, G], F32, tag="gates", bufs=2)
                    for d in (0, 1):
                        grows = slice(32 * d, 32 * d + 32)
                        for k in (0, 1):
                            if l == 0:
                                t_prev = (s - 1) if d == 0 else T - s
                                lhsT_h = h0T_r[:, k, d, t_prev, :]
                            else:
                                lhsT_h = hT_prev[:, k * 64 + 32 * d : k * 64 + 32 * d + 32]
                            for nh in (0, 1):
                                nc.tensor.matmul(
                                    gates[grows, nh * HALF : (nh + 1) * HALF],
                                    _r(lhsT_h),
                                    _r(
                                        whh_sb[d][
                                            :,
                                            k * G + nh * HALF : k * G + (nh + 1) * HALF,
                                        ]
                                    ),
                                    start=(k == 0),
                                    stop=(k == 1),
                                    tile_position=(0, 32 * d),
                                )
                    gsum = sb.tile([64, G], F32, tag="gsum", bufs=2)
                    nc.vector.tensor_add(gsum[:], gates[:], xgs[:])
                else:
                    gsum = xgs

                ifo = sb.tile([64, 768], F32, tag="ifo")
                nc.scalar.activation(ifo[:], gsum[:, 0:768], AF.Sigmoid)
                gt = sb.tile([64, 256], F32, tag="gt")
                nc.scalar.activation(gt[:], gsum[:, 768:1024], AF.Tanh)
                ig = sb.tile([64, 256], F32, tag="ig")
                nc.vector.tensor_mul(ig[:], ifo[:, 0:256], gt[:])
                c_new = sb.tile([64, 256], F32, tag="c", bufs=2)
                if s == 0:
                    nc.vector.tensor_copy(c_new[:], ig[:])
                else:
                    fc = sb.tile([64, 256], F32, tag="fc")
                    nc.vector.tensor_mul(fc[:], ifo[:, 256:512], c_prev[:])
                    nc.vector.tensor_add(c_new[:], fc[:], ig[:])
                c_prev = c_new
                tct = sb.tile([64, 256], F32, tag="tct")
                nc.scalar.activation(tct[:], c_new[:], AF.Tanh)
                h = sb.tile([64, 256], F32R, tag="h", bufs=3)
                nc.vector.tensor_mul(h[:], ifo[:, 512:768], tct[:])

                t_f, t_b = s, T - 1 - s
                if l == 0:
                    for k in (0, 1):
                        trp = ps_t.tile([128, 64], F32R, tag="trp", bufs=2)
                        nc.tensor.transpose(
                            _r(trp[:]),
                            _r(h[:, k * 128 : (k + 1) * 128]),
                            _r(identT_sb[:]),
                        )
                        nc.vector.tensor_copy(
                            h0T_r[:, k, 0, t_f, :], trp[:, 0:32]
                        )
                        nc.vector.tensor_copy(
                            h0T_r[:, k, 1, t_b, :], trp[:, 32:64]
                        )
                else:
                    nc.sync.dma_start(out_d[t_f, :, 0:256], h[0:32, :].bitcast(F32))
                    nc.sync.dma_start(out_d[t_b, :, 256:512], h[32:64, :].bitcast(F32))
                    if s < T - 1:
                        hT_new = sb.tile([128, 128], BF16, tag="h1T", bufs=2)
                        for k in (0, 1):
                            trp = ps_t.tile([128, 64], F32R, tag="trp", bufs=2)
                            nc.tensor.transpose(
                                _r(trp[:]),
                                _r(h[:, k * 128 : (k + 1) * 128]),
                                _r(identT_sb[:]),
                            )
                            nc.vector.tensor_copy(
                                hT_new[:, k * 64 : (k + 1) * 64], trp[:]
                            )
                        hT_prev = hT_new

    nc.compile()
    return nc


def _prep_inputs(inputs):
    x = np.asarray(inputs["x"], dtype=np.float32)
    common = {}
    for l in (0, 1):
        for d, sfx in enumerate(("", "_reverse")):
            Wih = np.asarray(inputs[f"weight_ih_l{l}{sfx}"], dtype=np.float32)
            Whh = np.asarray(inputs[f"weight_hh_l{l}{sfx}"], dtype=np.float32)
            bsum = (
                np.asarray(inputs[f"bias_ih_l{l}{sfx}"], dtype=np.float32)
                + np.asarray(inputs[f"bias_hh_l{l}{sfx}"], dtype=np.float32)
            )
            wihT = np.ascontiguousarray(Wih.T[:, _PERM])  # [512, 1024]
            whhT = np.ascontiguousarray(Whh.T[:, _PERM])  # [256, 1024]
            common[f"wih{l}{d}"] = (
                wihT.reshape(4, 128, G).transpose(1, 0, 2).reshape(128, 4 * G)
            )
            common[f"whh{l}{d}"] = (
                whhT.reshape(2, 128, G).transpose(1, 0, 2).reshape(128, 2 * G)
            )
            common[f"bias{l}{d}"] = bsum[_PERM][None, :]
    common["ident4"] = np.tile(np.eye(32, dtype=np.float32), (4, 1))
    common["ones"] = np.ones((1, 128), dtype=np.float32)
    common["identT"] = np.eye(64, dtype=np.float32)
    import ml_dtypes
    bf = ml_dtypes.bfloat16
    dts = {"ident4": bf}
    for l in (0, 1):
        for d in (0, 1):
            dts[f"whh{l}{d}"] = bf
            if l == 1:
                dts[f"wih{l}{d}"] = bf
    common = {
        k: np.ascontiguousarray(v, dtype=dts.get(k, np.float32))
        for k, v in common.items()
    }

    in_maps = []
    for c in range(NCORES):
        xs = x[:, c * BC : (c + 1) * BC, :]  # [T, 32, 512]
        m = dict(common)
        m["xT0"] = np.ascontiguousarray(xs.transpose(2, 0, 1).reshape(CIN, T * BC))
        m["xT1"] = np.ascontiguousarray(
            xs[::-1].transpose(2, 0, 1).reshape(CIN, T * BC)
        )
        in_maps.append(m)
    return in_maps


def _get_program():
    if "prog" not in _CACHE:
        _CACHE["prog"] = _build()
    return _CACHE["prog"]


def kernel(**inputs):
    nc = _get_program()
    in_maps = _prep_inputs(inputs)
    res = bass_utils.run_bass_kernel_spmd(nc, in_maps, core_ids=list(range(NCORES)))
    out = np.empty((T, B, 2 * H), np.float32)
    for c in range(NCORES):
        out[:, c * BC : (c + 1) * BC, :] = res.results[c]["out"]
    return out
